# revision 79
# baseline (speedup 1.0000x reference)
"""CQT (constant-Q transform) + amplitude_to_db kernel for Trainium2.

Full-input contract: kernel(x) takes x [32, 64000] f32 and returns
[32, 84, 126] f32, matching:

    frames = pad(x, n_fft//2)[:, t*HOP + n]          # [B, 126, 16384]
    cr/ci  = frames @ Kr.T / Ki.T                    # [B, 84, 126]
    mag    = sqrt(cr^2 + ci^2)
    out    = amplitude_to_db(mag, ref=max per item, amin=1e-5, top_db=80)

Sharding: pure data parallelism — 4 batch items per NeuronCore on 8 cores.

Per-core compute (v2 — hybrid fp16/fp8 DoubleRow):
  * The frame/filter contraction is one big matmul with K = n_fft = 16384,
    contracted in 128-row chunks. The frame matrix is never materialized:
    padded x stored column-major in SBUF ([128, 628] with x_cm[p,f] =
    xp[f*128+p]) makes chunk c of frames^T a strided AP view
    x_cm[:, c : c+501 : 4] (HOP=512 = 4*128).
  * CQT kernels are centered Hann-windowed exponentials: window ENERGY
    concentrates in the central K-chunks. Hybrid precision: the 12 central
    chunks [58,70) (+ the short B-group bins 64..83) run in fp16; the 78
    outer chunks run as 39 fp8e4m3 DoubleRow pair-matmuls (2 chunks / pass,
    2x PE rate). Simulated end-to-end rel_l2 = 5.6e-3 (gate 2e-2).
  * Scales: fp16 path (w*2^14, x*2^7), fp8 path (w*2^16, x*2^5) — equal
    products 2^21, so contributions add consistently in PSUM and the scale
    cancels in ref-normalized dB; only the amin clamp constant is scaled.
  * Tapered PE warmup (fp32 then fp16 junk matmuls) runs from block start
    until the first data lands so the HAM clock boost (2.4 GHz after ~3-5us
    of sustained activity) arrives as early as possible.
  * dB epilogue: squares split scalar/vector, single DVE clamp, max-reduce +
    GpSimd partition all-reduce for the per-item ref, Ln on ACT, per-item
    (lnm - lnr)*10/ln10 on DVE, per-item output DMAs from 4 idle engines.
"""

import numpy as np
import ml_dtypes
import bass_rust

import concourse.bass as bass
import concourse.mybir as mybir
from concourse import bacc
from concourse import bass_isa
from concourse.bass_utils import run_bass_kernel_spmd

# ---- problem constants (hardcoded; must match the reference) ----
SR = 22050
HOP = 512
N_BINS = 84
BPO = 12
FMIN = 32.70319566257483
AMIN = 1e-5
TOP_DB = 80.0
B = 32
N_SAMP = 64000
N_CORES = 8
NI = B // N_CORES            # items per core = 4
T = 1 + N_SAMP // HOP        # 126 frames
DB_SCALE = 10.0 / np.log(10.0)  # 20*log10(mag) == DB_SCALE * ln(mag^2)

P = 128
SPLIT_BIN = 64               # group A: bins [0,64), group B: bins [64,84)
NB_BINS = N_BINS - SPLIT_BIN  # 20
MB = 64                      # group B stationary width (re at 0:20, im at 32:52)

# precision scales: fp16 (w,x) and fp8 (w,x); products must match (2^21)
S_W16 = 2.0 ** 14
S_X16 = 2.0 ** 7
S_W8 = 2.0 ** 16
S_X8 = 2.0 ** 5
AMIN_M2 = float(AMIN * 2.0 ** 21) ** 2  # amin^2 in scaled-m2 units

HALF16 = 6                   # central fp16 chunk half-width around chunk 64

F16_NP = np.float16
F8_NP = ml_dtypes.float8_e4m3


def _build_cqt_kernels():
    """Same construction as the reference (nnAudio-style direct CQT bank)."""
    Q = 1.0 / (2.0 ** (1.0 / BPO) - 1.0)
    freqs = FMIN * 2.0 ** (np.arange(N_BINS) / BPO)
    lengths = np.ceil(Q * SR / freqs).astype(int)
    n_fft = int(2 ** np.ceil(np.log2(lengths.max())))
    K = np.zeros((N_BINS, n_fft), dtype=np.complex128)
    for k in range(N_BINS):
        L = int(lengths[k])
        t = np.arange(L) - (L - 1) / 2.0
        kern = np.hanning(L) * np.exp(2j * np.pi * freqs[k] * t / SR)
        kern /= np.abs(kern).sum()
        kern /= np.sqrt(L)
        s = (n_fft - L) // 2
        K[k, s:s + L] = kern
    return K.real.astype(np.float32), K.imag.astype(np.float32), n_fft


def _chunk_range(Kr, Ki, bins):
    nz = (np.abs(Kr[bins]).max(axis=0) + np.abs(Ki[bins]).max(axis=0)) > 0
    idx = np.nonzero(nz)[0]
    return int(idx[0]) // P, int(idx[-1]) // P + 1


Kr, Ki, N_FFT = _build_cqt_kernels()
PAD = N_FFT // 2
FW = (N_SAMP + 2 * PAD) // P      # 628 free-dim width of column-major xp
QW = FW // 4                      # 157
NT = NI * T                       # 504
assert (N_SAMP + 2 * PAD) % P == 0 and HOP == 4 * P
# fp16 x only needs the q-columns its chunks (58..69 + B 62..65) touch:
# q0 in [14,17] -> columns [14, 144) per phase
Q16_LO = 14
Q16_W = 130

_A0, _A1 = _chunk_range(Kr, Ki, range(0, SPLIT_BIN))          # [19, 109)
_B0, _B1 = _chunk_range(Kr, Ki, range(SPLIT_BIN, N_BINS))     # [62, 66)
C0, C1 = 64 - HALF16, 64 + HALF16                             # fp16 central

CH16 = [c for c in range(_A0, _A1) if C0 <= c < C1]           # 12 singles
OUTER = [c for c in range(_A0, _A1) if not (C0 <= c < C1)]    # 78 chunks
CHUNKS_B = list(range(_B0, _B1))                              # 4 singles

# fp8 DoubleRow pairs: same-phase q-adjacent pairs; odd leftovers pair
# cross-phase (any two chunks give one fixed k-tile stride in the x layout)
PAIRS = []
_leftover = []
for r in range(4):
    lst = [c for c in OUTER if c % 4 == r]
    for j in range(len(lst) // 2):
        PAIRS.append((lst[2 * j], lst[2 * j + 1]))
    if len(lst) % 2:
        _leftover.append(lst[-1])
assert len(_leftover) % 2 == 0
for j in range(len(_leftover) // 2):
    PAIRS.append((_leftover[2 * j], _leftover[2 * j + 1]))
N_PAIRS = len(PAIRS)          # 39

def _need_phase(op):
    kind, a = op
    if kind == "P8":
        return max(a[0] % 4, a[1] % 4)
    return a % 4

# schedule: fp8 pairs + fp16 singles (A central + B), ordered by the phase
# round in which their x data lands. Round 0 leads with fp8 pairs (their x
# phase + first weight slab is the smallest leading DMA); later rounds lead
# with the B16 single so psB finalizes at round-3 start and the B epilogue
# overlaps the remaining A matmuls.
# All fp8 pairs first (phase-ordered for x8 streaming), then the fp16 work:
# B16 singles (so psB finalizes ~2.5us before the last matmul and the B
# epilogue overlaps the S16 tail), then the A fp16 singles. This delays
# every x16/wb/wa16 DMA need by ~6us, freeing the HBM bandwidth at the
# start for the fp8 weight stream that feeds the PE at its consumption rate.
SCHEDULE = []
for r in range(4):
    SCHEDULE += [("P8", p) for p in PAIRS if _need_phase(("P8", p)) == r]
SCHEDULE += [("B16", c) for c in sorted(CHUNKS_B, key=lambda c: c % 4)]
SCHEDULE += [("S16", c) for c in CH16]

SCHED_P8 = [op for op in SCHEDULE if op[0] == "P8"]
SCHED_S16 = [op for op in SCHEDULE if op[0] == "S16"]
N_A_OPS = N_PAIRS + len(CH16)            # ops accumulating into psA
N_B_OPS = len(CHUNKS_B)

# weight slab splits (in schedule-order units). wa8 slabs alternate between
# the two HW-DGE queues (sync/scalar) in fine grains so the stream keeps up
# with the PE's ~154 GB/s weight consumption; gpsimd's slower SW-DGE queue
# only carries the small late-needed wa16/wb pieces.
SLAB8 = [3, 4, 5, 6, 7, 8, 6]            # pairs per wa8 dram slab
assert sum(SLAB8) == N_PAIRS
SLAB16 = [3, 3, 6]                       # singles per wa16 dram slab
assert sum(SLAB16) == len(CH16)


def _slab_of(sizes, j):
    off = 0
    for s, sz in enumerate(sizes):
        if j < off + sz:
            return s, j - off
        off += sz
    raise IndexError(j)


SLAB8_OFF = np.cumsum([0] + SLAB8).tolist()
SLAB16_OFF = np.cumsum([0] + SLAB16).tolist()


def _pack_weights():
    KrT = (Kr.T * S_W16).astype(np.float32)   # [N_FFT, 84] scaled fp16 path
    KiT = (Ki.T * S_W16).astype(np.float32)
    KrT8 = (Kr.T * S_W8).astype(np.float32)
    KiT8 = (Ki.T * S_W8).astype(np.float32)

    def a_cols(c, src_r, src_i):
        w = np.zeros((P, P), np.float32)
        w[:, :SPLIT_BIN] = src_r[c * P:(c + 1) * P, :SPLIT_BIN]
        w[:, SPLIT_BIN:] = src_i[c * P:(c + 1) * P, :SPLIT_BIN]
        return w

    wa8 = np.zeros((P, N_PAIRS * 2 * P), np.float32)
    for j, (_, (c1, c2)) in enumerate(SCHED_P8):
        wa8[:, j * 2 * P: j * 2 * P + P] = a_cols(c1, KrT8, KiT8)
        wa8[:, j * 2 * P + P: (j + 1) * 2 * P] = a_cols(c2, KrT8, KiT8)

    wa16 = np.zeros((P, len(CH16) * P), np.float32)
    for j, (_, c) in enumerate(SCHED_S16):
        wa16[:, j * P:(j + 1) * P] = a_cols(c, KrT, KiT)

    wb = np.zeros((P, len(CHUNKS_B) * MB), np.float32)
    for j, c in enumerate(CHUNKS_B):
        wb[:, j * MB: j * MB + NB_BINS] = KrT[c * P:(c + 1) * P, SPLIT_BIN:]
        wb[:, j * MB + 32: j * MB + 32 + NB_BINS] = KiT[c * P:(c + 1) * P,
                                                        SPLIT_BIN:]
    return wa8.astype(F8_NP), wa16.astype(F16_NP), wb.astype(F16_NP)


WA8, WA16, WB = _pack_weights()


def _chunk_off(c):
    """free-dim element offset of chunk c's first rhs element in the
    item-innermost xt layout (free index = r*QW*NI + q*NI + i): every
    chunk's 504 rhs columns (frame-major, item-minor) are contiguous."""
    return (c % 4) * NI * QW + (c // 4) * NI


def build_program():
    nc = bacc.Bacc("TRN2", target_bir_lowering=False, debug=False,
                   enable_asserts=True)
    f16 = mybir.dt.float16
    f8 = mybir.dt.float8e4
    f32 = mybir.dt.float32

    x8_in = nc.dram_tensor("x8_in", [4, P, NI * QW], f8,
                           kind="ExternalInput").ap()
    x16_in = nc.dram_tensor("x16_in", [4, P, NI * Q16_W], f16,
                            kind="ExternalInput").ap()
    wa8_in = nc.dram_tensor("wa8_in", [P, N_PAIRS * 2 * P], f8,
                            kind="ExternalInput").ap()
    wa16_in = nc.dram_tensor("wa16_in", [P, len(CH16) * P], f16,
                             kind="ExternalInput").ap()
    wb_in = nc.dram_tensor("wb_in", [P, len(CHUNKS_B) * MB], f16,
                           kind="ExternalInput").ap()
    out = nc.dram_tensor("out", [N_BINS, NI, T], f32, kind="ExternalOutput").ap()

    xt8_h = nc.alloc_sbuf_tensor("xt8", [P, NI * FW], f8)
    xt8 = xt8_h.ap()
    xt16 = nc.alloc_sbuf_tensor("xt16", [P, 4 * NI * Q16_W], f16).ap()
    wa8s = [nc.alloc_sbuf_tensor(f"wa8_{s}", [P, SLAB8[s] * 2 * P], f8).ap()
            for s in range(len(SLAB8))]
    wa16s = [nc.alloc_sbuf_tensor(f"wa16_{s}", [P, SLAB16[s] * P], f16).ap()
             for s in range(len(SLAB16))]
    wbt = nc.alloc_sbuf_tensor("wbt", [P, len(CHUNKS_B) * MB], f16).ap()
    bf16 = mybir.dt.bfloat16
    junk16 = nc.alloc_sbuf_tensor("junk16", [P, 512], f16).ap()
    m2 = nc.alloc_sbuf_tensor("m2", [N_BINS, NT], bf16).ap()
    tmp = nc.alloc_sbuf_tensor("tmp", [N_BINS, NT], bf16).ap()
    r1 = nc.alloc_sbuf_tensor("r1", [N_BINS, NI], f32).ap()
    rall = nc.alloc_sbuf_tensor("rall", [N_BINS, NI], f32).ap()
    lnm = nc.alloc_sbuf_tensor("lnm", [N_BINS, NT], f32).ap()
    lnr = nc.alloc_sbuf_tensor("lnr", [N_BINS, NI], f32).ap()
    db = nc.alloc_sbuf_tensor("db", [N_BINS, NT], f32).ap()
    lnwarm = nc.alloc_sbuf_tensor("lnwarm", [1, 2], f32).ap()

    psW = nc.alloc_psum_tensor("psW", [P, 504], f32).ap()
    psA = nc.alloc_psum_tensor("psA", [P, NT], f32).ap()
    psB = nc.alloc_psum_tensor("psB", [MB, NT], f32).ap()

    # one semaphore per input DMA: per-engine HWDGE round-robins dma_starts
    # over several hardware queues, so completion order on a shared counter
    # is not guaranteed
    s_x8 = [nc.alloc_semaphore(f"s_x8_{r}") for r in range(4)]
    s_x16 = [nc.alloc_semaphore(f"s_x16_{r}") for r in range(4)]
    s_w8 = [nc.alloc_semaphore(f"s_w8_{s}") for s in range(len(SLAB8))]
    s_w16 = [nc.alloc_semaphore(f"s_w16_{s}") for s in range(len(SLAB16))]
    s_wb = nc.alloc_semaphore("s_wb")
    s_mi = nc.alloc_semaphore("s_mi")     # junk memset done
    s_pe = nc.alloc_semaphore("s_pe")     # 1 = psB final, 2 = psA final
    s_a = nc.alloc_semaphore("s_a")       # ACT epilogue steps
    s_v = nc.alloc_semaphore("s_v")       # DVE epilogue steps
    s_g2 = nc.alloc_semaphore("s_g2")     # gpsimd all-reduce done
    s_out = nc.alloc_semaphore("s_out")   # 4 output DMAs x 16

    def rhs16(c):
        o = (c % 4) * NI * Q16_W + (c // 4 - Q16_LO) * NI
        return xt16[:, o: o + NT]

    def rhs8_pair(c1, c2):
        o1 = _chunk_off(c1)
        d = _chunk_off(c2) - o1
        assert d > 0
        return bass_rust.AP(xt8_h, o1, [[NI * FW, P], [d, 2], [1, NT]])

    Ln = mybir.ActivationFunctionType.Ln
    Square = mybir.ActivationFunctionType.Square
    DR = mybir.MatmulPerfMode.DoubleRow
    # m2 free order is (t, i)-interleaved. The Ln reads it via [t][i] dims
    # and writes lnm ITEM-MAJOR through a transposing output AP, so the db
    # ops read contiguous slices.
    m2_ti = m2.rearrange("p (t i) -> p t i", t=T, i=NI)
    m2_it2 = m2.rearrange("p (t i) -> p i t", t=T, i=NI)
    lnm_im = lnm.rearrange("p (i t) -> p t i", i=NI, t=T)
    out_flat = out.rearrange("k i t -> k (i t)")   # [84, 504], item-major

    # no_gpsimd_drain: skip the ~1.5us GpSimd DGE-drain at block exit — the
    # s_out>=64 wait already guarantees every output DMA completed.
    with nc.Block(no_gpsimd_drain=True) as block:

        def slab8_dma(eng, s):
            eng.dma_start(wa8s[s][:],
                          wa8_in[:, SLAB8_OFF[s] * 2 * P:SLAB8_OFF[s + 1] * 2 * P]
                          ).then_inc(s_w8[s], 16)

        def slab16_dma(eng, s):
            eng.dma_start(wa16s[s][:],
                          wa16_in[:, SLAB16_OFF[s] * P:SLAB16_OFF[s + 1] * P]
                          ).then_inc(s_w16[s], 16)

        QW16 = NI * Q16_W

        # An engine's queued dma_starts round-robin onto parallel HW rings
        # that SHARE bandwidth — issue order alone doesn't prioritize. Flow
        # control: gate later prefetches on earlier completions so only the
        # soon-needed transfers are in flight.
        @block.sync
        def _(sync):
            sync.dma_start(xt8[:, 0:NI * QW], x8_in[0]).then_inc(s_x8[0], 16)
            sync.dma_start(xt8[:, NI * QW:2 * NI * QW], x8_in[1]
                           ).then_inc(s_x8[1], 16)
            sync.wait_ge(s_x8[0], 16)
            slab8_dma(sync, 3)
            sync.dma_start(xt8[:, 2 * NI * QW:3 * NI * QW], x8_in[2]
                           ).then_inc(s_x8[2], 16)
            sync.wait_ge(s_x8[1], 16)
            sync.dma_start(xt8[:, 3 * NI * QW:], x8_in[3]).then_inc(s_x8[3], 16)
            sync.wait_ge(s_w8[3], 16)
            sync.dma_start(xt16[:, 0:QW16], x16_in[0]).then_inc(s_x16[0], 16)
            sync.dma_start(xt16[:, 2 * QW16:3 * QW16], x16_in[2]
                           ).then_inc(s_x16[2], 16)
            # output items 2+3 in one clean 2D DMA (bin-major dram layout)
            sync.wait_ge(s_v, 6)
            sync.dma_start(out_flat[:, 2 * T:], db[:, 2 * T:]
                           ).then_inc(s_out, 16)
            sync.wait_ge(s_out, 32)

        @block.scalar
        def _(scalar):
            slab8_dma(scalar, 0)
            slab8_dma(scalar, 1)
            scalar.wait_ge(s_w8[0], 16)
            slab8_dma(scalar, 4)
            scalar.wait_ge(s_w8[1], 16)
            slab8_dma(scalar, 5)
            scalar.wait_ge(s_w8[4], 16)
            slab8_dma(scalar, 6)
            scalar.dma_start(xt16[:, QW16:2 * QW16], x16_in[1]
                             ).then_inc(s_x16[1], 16)
            scalar.wait_ge(s_w8[5], 16)
            scalar.dma_start(xt16[:, 3 * QW16:], x16_in[3]
                             ).then_inc(s_x16[3], 16)
            # preload BOTH table slots (Ln set + Square set) while DMAs fly
            scalar.activation(lnwarm[:, 0:1], nc.const_aps.tensor(1.0, (1, 1)),
                              Ln)
            scalar.activation(lnwarm[:, 1:2], nc.const_aps.tensor(1.0, (1, 1)),
                              Square)
            # B epilogue (psB final before the S16 tail); squares land at
            # matching base partitions for the DVE combine
            scalar.wait_ge(s_pe, 1)
            scalar.activation(m2[SPLIT_BIN:], psB[:NB_BINS], Square
                              ).then_inc(s_a)
            scalar.activation(tmp[SPLIT_BIN:], psB[32:32 + NB_BINS], Square
                              ).then_inc(s_a)
            scalar.wait_ge(s_pe, 2)
            scalar.activation(m2[:SPLIT_BIN], psA[:SPLIT_BIN], Square
                              ).then_inc(s_a)
            scalar.activation(tmp[:SPLIT_BIN], psA[SPLIT_BIN:], Square
                              ).then_inc(s_a)
            scalar.wait_ge(s_v, 1)          # m2c complete
            scalar.activation(lnm_im, m2_ti, Ln).then_inc(s_a)
            scalar.wait_ge(s_g2, 1)
            scalar.activation(lnr[:], rall[:], Ln).then_inc(s_a)
            # output items 0+1 (scalar is the slowest issuer -> earlier pair)
            scalar.wait_ge(s_v, 4)
            scalar.dma_start(out_flat[:, 0:2 * T], db[:, 0:2 * T]
                             ).then_inc(s_out, 16)

        @block.gpsimd
        def _(gpsimd):
            slab8_dma(gpsimd, 2)
            gpsimd.dma_start(wbt[:], wb_in).then_inc(s_wb, 16)
            gpsimd.wait_ge(s_w8[2], 16)
            slab16_dma(gpsimd, 0)
            slab16_dma(gpsimd, 1)
            gpsimd.wait_ge(s_w16[0], 16)
            slab16_dma(gpsimd, 2)
            gpsimd.wait_ge(s_v, 2)
            gpsimd.partition_all_reduce(rall[:], r1[:], channels=N_BINS,
                                        reduce_op=bass_isa.ReduceOp.max
                                        ).then_inc(s_g2, 1)

        @block.vector
        def _(vector):
            vector.memset(junk16[:], 1.0).then_inc(s_mi, 1)
            # combine squares with the amin clamp folded into the add:
            # m2c = max(im^2, amin^2) + re^2  (differs from max(m2, amin^2)
            # only within epsilon of the clamp line — negligible here)
            vector.wait_ge(s_a, 2)
            vector.scalar_tensor_tensor(m2[SPLIT_BIN:], tmp[SPLIT_BIN:],
                                        AMIN_M2, m2[SPLIT_BIN:],
                                        mybir.AluOpType.max,
                                        mybir.AluOpType.add)
            vector.wait_ge(s_a, 4)          # both A squares done
            vector.scalar_tensor_tensor(m2[:SPLIT_BIN], tmp[:SPLIT_BIN],
                                        AMIN_M2, m2[:SPLIT_BIN],
                                        mybir.AluOpType.max,
                                        mybir.AluOpType.add)
            vector.drain().then_inc(s_v, 1)
            vector.tensor_reduce(r1[:], m2_it2,
                                 axis=mybir.AxisListType.X,
                                 op=mybir.AluOpType.max)
            vector.drain().then_inc(s_v, 1)
            vector.wait_ge(s_a, 6)          # lnm + lnr done
            for i in range(NI):
                vector.tensor_scalar(db[:, i * T:(i + 1) * T],
                                     lnm[:, i * T:(i + 1) * T],
                                     lnr[:, i:i + 1], float(DB_SCALE),
                                     mybir.AluOpType.subtract,
                                     mybir.AluOpType.mult)
                vector.drain().then_inc(s_v, 1)

        @block.tensor
        def _(tensor):
            tensor.wait_ge(s_mi, 1)
            # tapered warmup: continuous K=128 PE activity with NONZERO data
            # (the HAM activity monitor tracks real switching activity; junk
            # of zeros does not trigger the 2.4 GHz boost) from block start
            # until the first data lands
            for n in (504, 504, 504, 504, 504, 252, 252, 126, 126):
                tensor.matmul(psW[:, :n], lhsT=junk16[:, :P],
                              rhs=junk16[:, :n], start=True, stop=True)
            waited = set()

            def need(sem):
                if id(sem) not in waited:
                    tensor.wait_ge(sem, 16)
                    waited.add(id(sem))

            na = nb = n8 = n16 = 0
            for kind, a in SCHEDULE:
                if kind == "P8":
                    c1, c2 = a
                    need(s_x8[c1 % 4])
                    need(s_x8[c2 % 4])
                    s, o = _slab_of(SLAB8, n8)
                    need(s_w8[s])
                    tensor.matmul(
                        psA, lhsT=wa8s[s][:, o * 2 * P:(o + 1) * 2 * P]
                        .rearrange("p (k m) -> p k m", k=2),
                        rhs=rhs8_pair(c1, c2), perf_mode=DR,
                        start=(na == 0), stop=(na == N_A_OPS - 1),
                        skip_group_check=True)
                    n8 += 1
                    na += 1
                elif kind == "S16":
                    c = a
                    need(s_x16[c % 4])
                    s, o = _slab_of(SLAB16, n16)
                    need(s_w16[s])
                    tensor.matmul(psA, lhsT=wa16s[s][:, o * P:(o + 1) * P],
                                  rhs=rhs16(c), start=(na == 0),
                                  stop=(na == N_A_OPS - 1),
                                  skip_group_check=True)
                    n16 += 1
                    na += 1
                else:  # B16
                    c = a
                    j = CHUNKS_B.index(c)
                    need(s_x16[c % 4])
                    need(s_wb)
                    tensor.matmul(psB, lhsT=wbt[:, j * MB:(j + 1) * MB],
                                  rhs=rhs16(c), start=(nb == 0),
                                  stop=(nb == N_B_OPS - 1),
                                  skip_group_check=True)
                    nb += 1
                    if nb == N_B_OPS:
                        tensor.drain().then_inc(s_pe, 1)
                if na == N_A_OPS:
                    tensor.drain().then_inc(s_pe, 1)
                    na += 1  # fire once

    nc.compile()
    return nc


def pack_x(x):
    """x [B, 64000] f32 -> per-core phase-major, item-innermost packs:
    pack[r, p, q*NI + i] = xp[item i, (4q+r)*128 + p]. The fp16 pack only
    carries the q-column window [Q16_LO, Q16_LO+Q16_W) its chunks touch."""
    xp = np.pad(np.asarray(x, dtype=np.float32), ((0, 0), (PAD, PAD)))
    x_cm = xp.reshape(B, FW // 4, 4, P).transpose(0, 3, 2, 1)  # [B,128,4,157]
    p8, p16 = [], []
    for core in range(N_CORES):
        blk = x_cm[core * NI:(core + 1) * NI]           # [NI, 128, 4, 157]
        arr = blk.transpose(2, 1, 3, 0)                 # [4, 128, 157, NI]
        p8.append(np.ascontiguousarray(
            arr.reshape(4, P, QW * NI) * S_X8).astype(F8_NP))
        a16 = arr[:, :, Q16_LO:Q16_LO + Q16_W, :].reshape(4, P, Q16_W * NI)
        p16.append(np.ascontiguousarray(a16 * S_X16).astype(F16_NP))
    return p8, p16


_PROGRAM = None


def _get_program():
    global _PROGRAM
    if _PROGRAM is None:
        _PROGRAM = build_program()
    return _PROGRAM


def run(x, **spmd_kwargs):
    """Run on 8 NeuronCores; returns (output [32, 84, 126] f32, results)."""
    nc = _get_program()
    p8, p16 = pack_x(x)
    in_maps = [{"x8_in": p8[i], "x16_in": p16[i], "wa8_in": WA8,
                "wa16_in": WA16, "wb_in": WB} for i in range(N_CORES)]
    res = run_bass_kernel_spmd(nc, in_maps, core_ids=list(range(N_CORES)),
                               **spmd_kwargs)
    out = np.concatenate([res.results[i]["out"].transpose(1, 0, 2)
                          for i in range(N_CORES)], axis=0)
    return np.ascontiguousarray(out.astype(np.float32)), res


def kernel(x):
    return run(x)[0]
